# revision 23
# baseline (speedup 1.0000x reference)
"""Multi-head self-attention with RoPE (causal) on 8 Trainium2 NeuronCores.

Sharding: core c -> batch b = c//4, head-group g = c%4 (heads 4g..4g+3).
Each core computes a partial output x[b] @ block of Wo; host sums the 4
partials per batch.

Per-core layout strategy (all matmuls in float32r, fp32 accumulation):
  - x is fed transposed (xT [1024, 2048]); q,k are produced directly in
    transposed layout qT/kT [256 dims, 2048 seq] (dims on partitions).
  - RoPE applied in transposed layout: pair-swap via DVE stream_shuffle,
    combine with host-precomputed cos / sign-folded sin tables.
  - scores computed transposed (scoresT [sk, sq]) so softmax's key-sum is a
    matmul reduction: a ones-column appended to v makes the PV matmul emit
    the softmax denominator as out row 64.
  - exp on ScalarE with scale=1/8 (the 1/sqrt(d_k)); causal masking by
    computing only sq >= 128*t_sk per key tile + one affine_select zeroing
    of the diagonal 128x128 block after exp.
"""

import ml_dtypes
import numpy as np

import concourse.bass as bass
import concourse.mybir as mybir
import concourse.tile as tile
from concourse import bacc
from concourse import library_config
from concourse.bass_utils import run_bass_kernel_spmd

F32 = mybir.dt.float32
F32R = mybir.dt.float32r
BF16 = mybir.dt.bfloat16

D = 1024          # d_model
NH = 16           # total heads
DK = 64           # head dim
S = 2048          # seq len
B = 2             # batch
THETA = 10000.0
HPC = 4           # heads per core
DPC = HPC * DK    # dims per core = 256
N_CORES = 8

SWAP_MASK = [(i ^ 1) for i in range(32)]  # pair-swap within 32-lane groups


def _mm(nc, out, lhsT, rhs, start, stop):
    nc.tensor.matmul(out, lhsT, rhs, start=start, stop=stop)


def _emit(tc, aps):
    nc = tc.nc
    xT, wq, wk, wv, wo, cosc, sinc, outp = (
        aps["xT"], aps["wqT"], aps["wkT"], aps["wvT"], aps["woT"],
        aps["cosT"], aps["sinT"], aps["out"],
    )
    AF = mybir.ActivationFunctionType
    OP = mybir.AluOpType

    with tc.tile_pool(name="persist", bufs=1) as pp:
        # ---- persistent SBUF tensors ----
        qT_sb = pp.tile([128, 2, S], BF16, tag="qT")
        kT_sb = pp.tile([128, 2, S], BF16, tag="kT")
        v_sb = pp.tile([128, 16, HPC, DK + 1], BF16, tag="v")
        wo_sb = pp.tile([128, 2, D], BF16, tag="wo")
        attnT_sb = pp.tile([128, 2, S], BF16, tag="attnT")
        dmask_sb = pp.tile([128, 128], BF16, tag="dmask")
        _emit_phases(tc, aps, qT_sb, kT_sb, v_sb, wo_sb, attnT_sb, dmask_sb)


def _emit_phases(tc, aps, qT_sb, kT_sb, v_sb, wo_sb, attnT_sb, dmask_sb):
    nc = tc.nc
    xT, wq, wk, wv, wo, cosc, sinc, outp = (
        aps["xT"], aps["wqT"], aps["wkT"], aps["wvT"], aps["woT"],
        aps["cosT"], aps["sinT"], aps["out"],
    )
    AF = mybir.ActivationFunctionType
    OP = mybir.AluOpType

    with (
        tc.tile_pool(name="proj", bufs=1) as pj,
        tc.tile_pool(name="ropetmp", bufs=2) as rt,
        tc.tile_pool(name="psproj", bufs=4, space="PSUM") as psp,
    ):
        # ---- phase-P SBUF (freed after projections) ----
        xT_sb = pj.tile([128, 8, S], BF16, tag="xT")
        wq_sb = pj.tile([128, 8, DPC], BF16, tag="wq")
        wk_sb = pj.tile([128, 8, DPC], BF16, tag="wk")
        wv_sb = pj.tile([128, 8, DPC], BF16, tag="wv")
        cos_sb = pj.tile([128, S], F32, tag="cos")
        sin_sb = pj.tile([128, S], F32, tag="sin")

        # input DMAs
        for kt in range(8):
            nc.sync.dma_start(xT_sb[:, kt, :], xT[128 * kt:128 * (kt + 1), :])
        nc.sync.dma_start(wq_sb[:], wq.rearrange("(n p) m -> p n m", p=128))
        nc.sync.dma_start(wk_sb[:], wk.rearrange("(n p) m -> p n m", p=128))
        nc.sync.dma_start(wv_sb[:], wv.rearrange("(n p) m -> p n m", p=128))
        nc.sync.dma_start(wo_sb[:], wo.rearrange("(n p) m -> p n m", p=128))
        nc.sync.dma_start(cos_sb[:], cosc[:])
        nc.sync.dma_start(sin_sb[:], sinc[:])
        nc.sync.dma_start(dmask_sb[:], aps["dmask"][:])
        # ones column of v (denominator trick)
        nc.gpsimd.memset(v_sb[:, :, :, DK], 1.0)

        # ---- q/k projections (transposed out) + RoPE ----
        for w_sb, outT in ((wq_sb, qT_sb), (wk_sb, kT_sb)):
            for mt in range(2):
                for c in range(4):  # seq chunks of 512
                    ps = psp.tile([128, 512], F32, tag="proj")
                    for kt in range(8):
                        _mm(nc, ps[:],
                            w_sb[:, kt, 128 * mt:128 * (mt + 1)],
                            xT_sb[:, kt, 512 * c:512 * (c + 1)],
                            start=(kt == 0), stop=(kt == 7))
                    sl = slice(512 * c, 512 * (c + 1))
                    sw = rt.tile([128, 512], F32, tag="sw")
                    nc.vector.stream_shuffle(sw[:], ps[:], SWAP_MASK)
                    t1 = rt.tile([128, 512], BF16, tag="t1")
                    nc.vector.tensor_tensor(t1[:], ps[:], cos_sb[:, sl], OP.mult)
                    t2 = rt.tile([128, 512], BF16, tag="t2")
                    nc.gpsimd.tensor_tensor(t2[:], sw[:], sin_sb[:, sl], OP.mult)
                    nc.vector.tensor_tensor(outT[:, mt, sl], t1[:], t2[:], OP.add)

        # ---- v projection (normal layout, per-head 65-wide with ones col) ----
        for st in range(16):
            ps = psp.tile([128, DPC], F32, tag="vproj")
            for kt in range(8):
                _mm(nc, ps[:],
                    xT_sb[:, kt, 128 * st:128 * (st + 1)],
                    wv_sb[:, kt, :],
                    start=(kt == 0), stop=(kt == 7))
            nc.vector.tensor_copy(
                v_sb[:, st, :, 0:DK],
                ps[:].rearrange("p (h e) -> p h e", e=DK),
            )

    # ---- attention ----
    with (
        tc.tile_pool(name="attn", bufs=2) as pa,
        tc.tile_pool(name="exp", bufs=4) as pe,
        tc.tile_pool(name="dr", bufs=2, space="DRAM") as dr,
        tc.tile_pool(name="psattn", bufs=2, space="PSUM") as psa,
    ):
        for h in range(HPC):
            sub, ph = h % 2, h // 2
            prow = slice(64 * sub, 64 * (sub + 1))
            for H in range(2):
                q_hi = 1024 * (H + 1)
                at_ps = psa.tile([DK + 1, 1024], F32, tag="at")
                t_hi = 8 * (H + 1)  # exclusive
                for t in range(t_hi):
                    sq_lo = max(128 * t, 1024 * H)
                    L = q_hi - sq_lo
                    sc = psa.tile([128, 1024], F32, tag="sc")
                    off = 0
                    while off < L:
                        n = min(512, L - off)
                        _mm(nc, sc[:, off:off + n],
                            kT_sb[prow, ph, 128 * t:128 * (t + 1)],
                            qT_sb[prow, ph, sq_lo + off:sq_lo + off + n],
                            start=True, stop=True)
                        off += n
                    ex = pe.tile([128, 1024], BF16, tag="exp")
                    nc.scalar.activation(ex[:, 0:L], sc[:, 0:L], AF.Exp, scale=0.125)
                    if 128 * t >= 1024 * H:
                        # diagonal block: zero exp where local_sq < partition
                        nc.gpsimd.tensor_tensor(
                            ex[:, 0:128], ex[:, 0:128], dmask_sb[:], OP.mult)
                    # PV accumulation (+ denominator via ones column)
                    for ck in range(2):
                        c_lo, c_hi = 1024 * H + 512 * ck, 1024 * H + 512 * (ck + 1)
                        if sq_lo >= c_hi:
                            continue
                        lo = max(sq_lo, c_lo)
                        last_t = min(t_hi, (c_hi + 127) // 128) - 1
                        _mm(nc, at_ps[:, lo - 1024 * H:c_hi - 1024 * H],
                            v_sb[:, t, h, :],
                            ex[:, lo - sq_lo:c_hi - sq_lo],
                            start=(t == 0), stop=(t == last_t))
                # normalize: recip of denom row, broadcast, multiply
                rc = pa.tile([DK + 1, 1024], F32, tag="rc")
                nc.vector.reciprocal(rc[DK:DK + 1, :], at_ps[DK:DK + 1, :])
                rc_d = dr.tile([1, 1024], F32, tag="rcd")
                nc.sync.dma_start(rc_d[:], rc[DK:DK + 1, :])
                bc = pa.tile([DK, 1024], F32, tag="bc")
                nc.gpsimd.dma_start(
                    out=bc[:], in_=rc_d[:].partition_broadcast(DK))
                Hsl = slice(1024 * H, 1024 * (H + 1))
                if sub == 0:
                    nc.vector.tensor_tensor(
                        attnT_sb[0:64, ph, Hsl], at_ps[0:DK, :], bc[:], OP.mult)
                else:
                    tn = pa.tile([DK, 1024], BF16, tag="tn")
                    nc.vector.tensor_tensor(tn[:], at_ps[0:DK, :], bc[:], OP.mult)
                    nc.sync.dma_start(attnT_sb[64:128, ph, Hsl], tn[:])

        # ---- output projection; copy PSUM->SBUF (alternating engines), DMA out ----
        for st in range(16):
            for ncb in range(2):
                po = psa.tile([128, 512], F32, tag="sc")
                for kt2 in range(2):
                    _mm(nc, po[:, 0:512],
                        attnT_sb[:, kt2, 128 * st:128 * (st + 1)],
                        wo_sb[:, kt2, 512 * ncb:512 * (ncb + 1)],
                        start=(kt2 == 0), stop=(kt2 == 1))
                ob = pa.tile([128, 512], F32, tag="ob")
                if ncb == 0:
                    nc.scalar.copy(ob[:], po[:, 0:512])
                else:
                    nc.vector.tensor_copy(ob[:], po[:, 0:512])
                nc.sync.dma_start(
                    outp[128 * st:128 * (st + 1), 512 * ncb:512 * (ncb + 1)],
                    ob[:])


_CACHE = {}


def _build():
    if "nc" in _CACHE:
        return _CACHE["nc"], _CACHE["aps"]
    nc = bacc.Bacc("TRN2", target_bir_lowering=False, debug=False,
                   enable_asserts=False, num_devices=N_CORES)
    aps = {
        "xT": nc.dram_tensor("xT", [D, S], BF16, kind="ExternalInput").ap(),
        "wqT": nc.dram_tensor("wqT", [D, DPC], BF16, kind="ExternalInput").ap(),
        "wkT": nc.dram_tensor("wkT", [D, DPC], BF16, kind="ExternalInput").ap(),
        "wvT": nc.dram_tensor("wvT", [D, DPC], BF16, kind="ExternalInput").ap(),
        "woT": nc.dram_tensor("woT", [DPC, D], BF16, kind="ExternalInput").ap(),
        "cosT": nc.dram_tensor("cosT", [128, S], F32, kind="ExternalInput").ap(),
        "sinT": nc.dram_tensor("sinT", [128, S], F32, kind="ExternalInput").ap(),
        "dmask": nc.dram_tensor("dmask", [128, 128], BF16, kind="ExternalInput").ap(),
        "out": nc.dram_tensor("out", [S, D], F32, kind="ExternalOutput").ap(),
    }
    with tile.TileContext(nc) as tc:
        _emit(tc, aps)
    nc.compile()
    _CACHE["nc"], _CACHE["aps"] = nc, aps
    return nc, aps


def _host_tables():
    pos = np.arange(S, dtype=np.float64)
    freqs = THETA ** (-np.arange(0, DK, 2, dtype=np.float64) / DK)
    ang = pos[:, None] * freqs[None, :]          # [S, 32]
    cos64 = np.empty((64, S), np.float32)
    sin64 = np.empty((64, S), np.float32)
    cos64[0::2] = cos64[1::2] = np.cos(ang).T
    sin64[0::2] = -np.sin(ang).T
    sin64[1::2] = np.sin(ang).T
    return (np.ascontiguousarray(np.concatenate([cos64, cos64], axis=0)),
            np.ascontiguousarray(np.concatenate([sin64, sin64], axis=0)))


def make_in_maps(x, Wq, Wk, Wv, Wo):
    cosT, sinT = _host_tables()
    dmask = np.triu(np.ones((128, 128), ml_dtypes.bfloat16))  # keep sq >= sk
    xT = [np.ascontiguousarray(x[b].T.astype(ml_dtypes.bfloat16)) for b in range(B)]
    maps = []
    for c in range(N_CORES):
        b, g = c // 4, c % 4
        rows = slice(DPC * g, DPC * (g + 1))
        maps.append({
            "xT": xT[b],
            "wqT": np.ascontiguousarray(Wq[rows, :].T.astype(ml_dtypes.bfloat16)),
            "wkT": np.ascontiguousarray(Wk[rows, :].T.astype(ml_dtypes.bfloat16)),
            "wvT": np.ascontiguousarray(Wv[rows, :].T.astype(ml_dtypes.bfloat16)),
            "woT": np.ascontiguousarray(Wo[:, rows].T.astype(ml_dtypes.bfloat16)),
            "cosT": cosT,
            "sinT": sinT,
            "dmask": dmask,
        })
    return maps


def kernel(x, Wq, Wk, Wv, Wo, _trace=False, _tmpdir=None):
    x, Wq, Wk, Wv, Wo = (np.asarray(a, dtype=np.float32) for a in (x, Wq, Wk, Wv, Wo))
    nc, _ = _build()
    maps = make_in_maps(x, Wq, Wk, Wv, Wo)
    res = run_bass_kernel_spmd(nc, maps, core_ids=list(range(N_CORES)),
                               trace=_trace, tmpdir=_tmpdir)
    out = np.zeros((B, S, D), np.float32)
    for c in range(N_CORES):
        out[c // 4] += res.results[c]["out"]
    if _trace:
        kernel.last_results = res
    return out


# revision 39
# speedup vs baseline: 1.1491x; 1.1491x over previous
"""Multi-head self-attention with RoPE (causal) on 8 Trainium2 NeuronCores.

Sharding: core c -> batch b = c//4, head-group g = c%4 (heads 4g..4g+3).
Each core computes a partial output x[b] @ block of Wo; host sums the 4
partials per batch.

Per-core layout strategy (all matmuls in bf16, fp32 PSUM accumulation):
  - x is fed transposed (xT [1024, 2048]); q,k are produced directly in
    transposed layout qT/kT [256 dims, 2048 seq] (dims on partitions).
  - RoPE applied in transposed layout: pair-swap via DVE stream_shuffle,
    combine with host-precomputed cos / sign-folded sin tables.
  - scores computed transposed (scoresT [sk, sq]) so softmax's key-sum is a
    matmul reduction: a ones-column appended to v makes the PV matmul emit
    the softmax denominator as out row 64.
  - exp on ScalarE with scale=1/8 (the 1/sqrt(d_k)); causal masking by
    computing only sq >= 128*t_sk per key tile + one affine_select zeroing
    of the diagonal 128x128 block after exp.
"""

import ml_dtypes
import numpy as np

import concourse.bass as bass
import concourse.mybir as mybir
import concourse.tile as tile
from concourse import bacc
from concourse import library_config
from concourse.bass_utils import run_bass_kernel_spmd

F32 = mybir.dt.float32
F32R = mybir.dt.float32r
BF16 = mybir.dt.bfloat16

D = 1024          # d_model
NH = 16           # total heads
DK = 64           # head dim
S = 2048          # seq len
B = 2             # batch
THETA = 10000.0
HPC = 4           # heads per core
DPC = HPC * DK    # dims per core = 256
N_CORES = 8

SWAP_MASK = [(i ^ 1) for i in range(32)]  # pair-swap within 32-lane groups


def _mm(nc, out, lhsT, rhs, start, stop):
    nc.tensor.matmul(out, lhsT, rhs, start=start, stop=stop)


def _emit(tc, aps):
    nc = tc.nc
    xT, wq, wk, wv, wo, cosc, sinc, outp = (
        aps["xT"], aps["wqT"], aps["wkT"], aps["wvT"], aps["woT"],
        aps["cosT"], aps["sinT"], aps["out"],
    )
    AF = mybir.ActivationFunctionType
    OP = mybir.AluOpType

    with (
        tc.tile_pool(name="persist", bufs=1) as pp,
        tc.tile_pool(name="ropetmp", bufs=2) as rt,
        tc.tile_pool(name="attn", bufs=2) as pa,
        tc.tile_pool(name="exp", bufs=14) as pe,
        tc.tile_pool(name="psum", bufs=2, space="PSUM") as psa,
    ):
        # ---- persistent SBUF tensors ----
        qT_sb = pp.tile([128, 2, S], BF16, tag="qT")
        kT_sb = pp.tile([128, 2, S], BF16, tag="kT")
        v_sb = pp.tile([128, 16, HPC, DK + 1], BF16, tag="v")
        wo_sb = pp.tile([128, 2, D], BF16, tag="wo")
        attnT_sb = pp.tile([128, 2, S], BF16, tag="attnT")
        dmask_sb = pp.tile([128, 128], BF16, tag="dmask")
        xT_sb = pp.tile([128, 8, S], BF16, tag="xT")
        wq_sb = pp.tile([128, 8, DPC], BF16, tag="wq")
        wk_sb = pp.tile([128, 8, DPC], BF16, tag="wk")
        wv_sb = pp.tile([128, 8, DPC], BF16, tag="wv")
        cos_sb = pp.tile([128, S], F32, tag="cos")
        sin_sb = pp.tile([128, S], F32, tag="sin")

        # input DMAs, ordered so the first projection chunk unblocks ASAP:
        # wq fully, then xT's first 512 columns across all k-tiles
        def xT_dma(c):
            for kt in range(8):
                nc.sync.dma_start(
                    xT_sb[:, kt, 512 * c:512 * (c + 1)],
                    xT[128 * kt:128 * (kt + 1), 512 * c:512 * (c + 1)])
        for kt in range(8):
            nc.sync.dma_start(wq_sb[:, kt, :], wq[128 * kt:128 * (kt + 1), :])
        xT_dma(0)
        for kt in range(8):
            nc.sync.dma_start(wk_sb[:, kt, :], wk[128 * kt:128 * (kt + 1), :])
        xT_dma(1)
        nc.sync.dma_start(cos_sb[:], cosc[:])
        nc.sync.dma_start(sin_sb[:], sinc[:])
        xT_dma(2)
        xT_dma(3)
        nc.sync.dma_start(dmask_sb[:], aps["dmask"][:])
        for kt in range(8):
            nc.sync.dma_start(wv_sb[:, kt, :], wv[128 * kt:128 * (kt + 1), :])
        nc.sync.dma_start(wo_sb[:], wo.rearrange("(n p) m -> p n m", p=128))
        # ones column of v (denominator trick)
        nc.gpsimd.memset(v_sb[:, :, :, DK], 1.0)
        ones_sb = pp.tile([DK + 1, DK], BF16, tag="ones")
        nc.gpsimd.memset(ones_sb[:], 1.0)

        def qk_chunk(w_sb, outT, mt, c):
                ps = psa.tile([128, 512], F32, tag="pj", bufs=2)
                for kt in range(8):
                    _mm(nc, ps[:],
                        w_sb[:, kt, 128 * mt:128 * (mt + 1)],
                        xT_sb[:, kt, 512 * c:512 * (c + 1)],
                        start=(kt == 0), stop=(kt == 7))
                sl = slice(512 * c, 512 * (c + 1))
                sw = rt.tile([128, 512], F32, tag="sw")
                nc.vector.stream_shuffle(sw[:], ps[:], SWAP_MASK)
                t1 = rt.tile([128, 512], BF16, tag="t1")
                nc.vector.tensor_tensor(t1[:], ps[:], cos_sb[:, sl], OP.mult)
                t2 = rt.tile([128, 512], BF16, tag="t2")
                nc.gpsimd.tensor_tensor(t2[:], sw[:], sin_sb[:, sl], OP.mult)
                nc.vector.tensor_tensor(outT[:, mt, sl], t1[:], t2[:], OP.add)

        def v_proj():
            for st2 in range(8):
                ps = psa.tile([128, 512], F32, tag="pj", bufs=2)
                for half in range(2):
                    st = 2 * st2 + half
                    for kt in range(8):
                        _mm(nc, ps[:, DPC * half:DPC * (half + 1)],
                            xT_sb[:, kt, 128 * st:128 * (st + 1)],
                            wv_sb[:, kt, :],
                            start=(kt == 0), stop=(kt == 7))
                nc.vector.tensor_copy(
                    v_sb[:, 2 * st2:2 * st2 + 2, :, 0:DK],
                    ps[:].rearrange("p (s h e) -> p s h e", s=2, h=HPC),
                )

        def attention(h):
            sub, ph = h % 2, h // 2
            prow = slice(64 * sub, 64 * (sub + 1))
            for H in range(2):
                q_hi = 1024 * (H + 1)
                at_ps = psa.tile([DK + 1, 1024], F32, tag="at", bufs=1)
                t_hi = 8 * (H + 1)  # exclusive
                for t in range(t_hi):
                    sq_lo = max(128 * t, 1024 * H)
                    L = q_hi - sq_lo
                    sc = psa.tile([128, 1024], F32, tag="sc", bufs=2)
                    off = 0
                    while off < L:
                        n = min(512, L - off)
                        _mm(nc, sc[:, off:off + n],
                            kT_sb[prow, ph, 128 * t:128 * (t + 1)],
                            qT_sb[prow, ph, sq_lo + off:sq_lo + off + n],
                            start=True, stop=True)
                        off += n
                    ex = pe.tile([128, 1024], BF16, tag="exp")
                    nc.scalar.activation(ex[:, 0:L], sc[:, 0:L], AF.Exp, scale=0.125)
                    if 128 * t >= 1024 * H:
                        # diagonal block: zero exp where local_sq < partition
                        nc.gpsimd.tensor_tensor(
                            ex[:, 0:128], ex[:, 0:128], dmask_sb[:], OP.mult)
                    # PV accumulation (+ denominator via ones column)
                    for ck in range(2):
                        c_lo, c_hi = 1024 * H + 512 * ck, 1024 * H + 512 * (ck + 1)
                        if sq_lo >= c_hi:
                            continue
                        lo = max(sq_lo, c_lo)
                        last_t = min(t_hi, (c_hi + 127) // 128) - 1
                        _mm(nc, at_ps[:, lo - 1024 * H:c_hi - 1024 * H],
                            v_sb[:, t, h, :],
                            ex[:, lo - sq_lo:c_hi - sq_lo],
                            start=(t == 0), stop=(t == last_t))
                # normalize: recip of denom row, PE broadcast matmul, multiply
                rc = pa.tile([DK + 1, 1024], BF16, tag="rc")
                with nc.allow_low_precision(reason="bf16 softmax recip broadcast"):
                    nc.vector.reciprocal(rc[DK:DK + 1, :], at_ps[DK:DK + 1, :])
                ac = pa.tile([DK, 1024], BF16, tag="ac")
                nc.vector.tensor_copy(ac[:], at_ps[0:DK, :])
                bc_ps = psa.tile([DK, 1024], F32, tag="sc", bufs=2)
                for bh in range(2):
                    _mm(nc, bc_ps[:, 512 * bh:512 * (bh + 1)],
                        ones_sb[DK:DK + 1, :],
                        rc[DK:DK + 1, 512 * bh:512 * (bh + 1)],
                        start=True, stop=True)
                Hsl = slice(1024 * H, 1024 * (H + 1))
                if sub == 0:
                    nc.vector.tensor_tensor(
                        attnT_sb[0:64, ph, Hsl], ac[:], bc_ps[:], OP.mult)
                else:
                    tn = pa.tile([DK, 1024], BF16, tag="tn")
                    nc.vector.tensor_tensor(tn[:], ac[:], bc_ps[:], OP.mult)
                    nc.sync.dma_start(attnT_sb[64:128, ph, Hsl], tn[:])

        def out_proj():
            optags = [("pj", 2), ("sc", 2), ("at", 1)]
            for st in range(16):
                for ncb in range(2):
                    tg, bf = optags[(2 * st + ncb) % 3]
                    po = psa.tile([128, 512], F32, tag=tg, bufs=bf)
                    for kt2 in range(2):
                        _mm(nc, po[:, 0:512],
                            attnT_sb[:, kt2, 128 * st:128 * (st + 1)],
                            wo_sb[:, kt2, 512 * ncb:512 * (ncb + 1)],
                            start=(kt2 == 0), stop=(kt2 == 1))
                    ob = pa.tile([128, 512], F32, tag="ob", bufs=6)
                    if ncb == 0:
                        nc.scalar.copy(ob[:], po[:, 0:512])
                    else:
                        nc.vector.tensor_copy(ob[:], po[:, 0:512])
                    nc.sync.dma_start(
                        outp[128 * st:128 * (st + 1), 512 * ncb:512 * (ncb + 1)],
                        ob[:])

        # head-pair pipelined emission: attention on heads 0,1 overlaps
        # the projections for heads 2,3
        for c in range(4):
            qk_chunk(wq_sb, qT_sb, 0, c)
            qk_chunk(wk_sb, kT_sb, 0, c)
        v_proj()
        with tc.high_priority():
            attention(0)
            attention(1)
        for c in range(4):
            qk_chunk(wq_sb, qT_sb, 1, c)
            qk_chunk(wk_sb, kT_sb, 1, c)
        attention(3)
        attention(2)
        out_proj()


_CACHE = {}


def _build():
    if "nc" in _CACHE:
        return _CACHE["nc"], _CACHE["aps"]
    nc = bacc.Bacc("TRN2", target_bir_lowering=False, debug=False,
                   enable_asserts=False, num_devices=N_CORES)
    aps = {
        "xT": nc.dram_tensor("xT", [D, S], BF16, kind="ExternalInput").ap(),
        "wqT": nc.dram_tensor("wqT", [D, DPC], BF16, kind="ExternalInput").ap(),
        "wkT": nc.dram_tensor("wkT", [D, DPC], BF16, kind="ExternalInput").ap(),
        "wvT": nc.dram_tensor("wvT", [D, DPC], BF16, kind="ExternalInput").ap(),
        "woT": nc.dram_tensor("woT", [DPC, D], BF16, kind="ExternalInput").ap(),
        "cosT": nc.dram_tensor("cosT", [128, S], F32, kind="ExternalInput").ap(),
        "sinT": nc.dram_tensor("sinT", [128, S], F32, kind="ExternalInput").ap(),
        "dmask": nc.dram_tensor("dmask", [128, 128], BF16, kind="ExternalInput").ap(),
        "out": nc.dram_tensor("out", [S, D], F32, kind="ExternalOutput").ap(),
    }
    with tile.TileContext(nc) as tc:
        _emit(tc, aps)
    nc.compile()
    _CACHE["nc"], _CACHE["aps"] = nc, aps
    return nc, aps


def _host_tables():
    pos = np.arange(S, dtype=np.float64)
    freqs = THETA ** (-np.arange(0, DK, 2, dtype=np.float64) / DK)
    ang = pos[:, None] * freqs[None, :]          # [S, 32]
    cos64 = np.empty((64, S), np.float32)
    sin64 = np.empty((64, S), np.float32)
    cos64[0::2] = cos64[1::2] = np.cos(ang).T
    sin64[0::2] = -np.sin(ang).T
    sin64[1::2] = np.sin(ang).T
    return (np.ascontiguousarray(np.concatenate([cos64, cos64], axis=0)),
            np.ascontiguousarray(np.concatenate([sin64, sin64], axis=0)))


def make_in_maps(x, Wq, Wk, Wv, Wo):
    cosT, sinT = _host_tables()
    dmask = np.triu(np.ones((128, 128), ml_dtypes.bfloat16))  # keep sq >= sk
    xT = [np.ascontiguousarray(x[b].T.astype(ml_dtypes.bfloat16)) for b in range(B)]
    maps = []
    for c in range(N_CORES):
        b, g = c // 4, c % 4
        rows = slice(DPC * g, DPC * (g + 1))
        maps.append({
            "xT": xT[b],
            "wqT": np.ascontiguousarray(Wq[rows, :].T.astype(ml_dtypes.bfloat16)),
            "wkT": np.ascontiguousarray(Wk[rows, :].T.astype(ml_dtypes.bfloat16)),
            "wvT": np.ascontiguousarray(Wv[rows, :].T.astype(ml_dtypes.bfloat16)),
            "woT": np.ascontiguousarray(Wo[:, rows].T.astype(ml_dtypes.bfloat16)),
            "cosT": cosT,
            "sinT": sinT,
            "dmask": dmask,
        })
    return maps


def kernel(x, Wq, Wk, Wv, Wo, _trace=False, _tmpdir=None):
    x, Wq, Wk, Wv, Wo = (np.asarray(a, dtype=np.float32) for a in (x, Wq, Wk, Wv, Wo))
    nc, _ = _build()
    maps = make_in_maps(x, Wq, Wk, Wv, Wo)
    res = run_bass_kernel_spmd(nc, maps, core_ids=list(range(N_CORES)),
                               trace=_trace, tmpdir=_tmpdir)
    out = np.zeros((B, S, D), np.float32)
    for c in range(N_CORES):
        out[c // 4] += res.results[c]["out"]
    if _trace:
        kernel.last_results = res
    return out


# revision 41
# speedup vs baseline: 1.1663x; 1.0150x over previous
"""Multi-head self-attention with RoPE (causal) on 8 Trainium2 NeuronCores.

Sharding: core c -> batch b = c//4, head-group g = c%4 (heads 4g..4g+3).
Each core computes a partial output x[b] @ block of Wo; host sums the 4
partials per batch.

Per-core layout strategy (all matmuls in bf16, fp32 PSUM accumulation):
  - x is fed transposed (xT [1024, 2048]); q,k are produced directly in
    transposed layout qT/kT [256 dims, 2048 seq] (dims on partitions).
  - RoPE applied in transposed layout: pair-swap via DVE stream_shuffle,
    combine with host-precomputed cos / sign-folded sin tables.
  - scores computed transposed (scoresT [sk, sq]) so softmax's key-sum is a
    matmul reduction: a ones-column appended to v makes the PV matmul emit
    the softmax denominator as out row 64.
  - exp on ScalarE with scale=1/8 (the 1/sqrt(d_k)); causal masking by
    computing only sq >= 128*t_sk per key tile + one affine_select zeroing
    of the diagonal 128x128 block after exp.
"""

import ml_dtypes
import numpy as np

import concourse.bass as bass
import concourse.mybir as mybir
import concourse.tile as tile
from concourse import bacc
from concourse import library_config
from concourse.bass_utils import run_bass_kernel_spmd

F32 = mybir.dt.float32
F32R = mybir.dt.float32r
BF16 = mybir.dt.bfloat16

D = 1024          # d_model
NH = 16           # total heads
DK = 64           # head dim
S = 2048          # seq len
B = 2             # batch
THETA = 10000.0
HPC = 4           # heads per core
DPC = HPC * DK    # dims per core = 256
N_CORES = 8

SWAP_MASK = [(i ^ 1) for i in range(32)]  # pair-swap within 32-lane groups


def _mm(nc, out, lhsT, rhs, start, stop):
    nc.tensor.matmul(out, lhsT, rhs, start=start, stop=stop)


def _emit(tc, aps):
    nc = tc.nc
    xT, wq, wk, wv, wo, cosc, sinc, outp = (
        aps["xT"], aps["wqT"], aps["wkT"], aps["wvT"], aps["woT"],
        aps["cosT"], aps["sinT"], aps["out"],
    )
    AF = mybir.ActivationFunctionType
    OP = mybir.AluOpType

    with (
        tc.tile_pool(name="persist", bufs=1) as pp,
        tc.tile_pool(name="ropetmp", bufs=2) as rt,
        tc.tile_pool(name="attn", bufs=2) as pa,
        tc.tile_pool(name="exp", bufs=14) as pe,
        tc.tile_pool(name="psum", bufs=2, space="PSUM") as psa,
    ):
        # ---- persistent SBUF tensors ----
        qT_sb = pp.tile([128, 2, S], BF16, tag="qT")
        kT_sb = pp.tile([128, 2, S], BF16, tag="kT")
        v_sb = pp.tile([128, 16, HPC, DK + 1], BF16, tag="v")
        wo_sb = pp.tile([128, 2, D], BF16, tag="wo")
        attnT_sb = pp.tile([128, 2, S], BF16, tag="attnT")
        dmask_sb = pp.tile([128, 128], BF16, tag="dmask")
        xT_sb = pp.tile([128, 8, S], BF16, tag="xT")
        wq_sb = pp.tile([128, 8, DPC], BF16, tag="wq")
        wk_sb = pp.tile([128, 8, DPC], BF16, tag="wk")
        wv_sb = pp.tile([128, 8, DPC], BF16, tag="wv")
        cos_sb = pp.tile([128, S], F32, tag="cos")
        sin_sb = pp.tile([128, S], F32, tag="sin")

        # input DMAs, ordered so the first projection chunk unblocks ASAP:
        # wq fully, then xT's first 512 columns across all k-tiles
        def xT_dma(c):
            for kt in range(8):
                nc.sync.dma_start(
                    xT_sb[:, kt, 512 * c:512 * (c + 1)],
                    xT[128 * kt:128 * (kt + 1), 512 * c:512 * (c + 1)])
        def cs_dma(c):
            sl = slice(512 * c, 512 * (c + 1))
            nc.sync.dma_start(cos_sb[:, sl], cosc[:, sl])
            nc.sync.dma_start(sin_sb[:, sl], sinc[:, sl])
        for kt in range(8):
            nc.sync.dma_start(wq_sb[:, kt, :], wq[128 * kt:128 * (kt + 1), :])
        xT_dma(0)
        cs_dma(0)
        for kt in range(8):
            nc.sync.dma_start(wk_sb[:, kt, :], wk[128 * kt:128 * (kt + 1), :])
        xT_dma(1)
        cs_dma(1)
        xT_dma(2)
        cs_dma(2)
        xT_dma(3)
        cs_dma(3)
        nc.sync.dma_start(dmask_sb[:], aps["dmask"][:])
        for kt in range(8):
            nc.sync.dma_start(wv_sb[:, kt, :], wv[128 * kt:128 * (kt + 1), :])
        nc.sync.dma_start(wo_sb[:], wo.rearrange("(n p) m -> p n m", p=128))
        # ones column of v (denominator trick)
        nc.gpsimd.memset(v_sb[:, :, :, DK], 1.0)
        ones_sb = pp.tile([DK + 1, DK], BF16, tag="ones")
        nc.gpsimd.memset(ones_sb[:], 1.0)

        def qk_chunk(w_sb, outT, mt, c):
                ps = psa.tile([128, 512], F32, tag="pj", bufs=2)
                for kt in range(8):
                    _mm(nc, ps[:],
                        w_sb[:, kt, 128 * mt:128 * (mt + 1)],
                        xT_sb[:, kt, 512 * c:512 * (c + 1)],
                        start=(kt == 0), stop=(kt == 7))
                sl = slice(512 * c, 512 * (c + 1))
                sw = rt.tile([128, 512], F32, tag="sw")
                nc.vector.stream_shuffle(sw[:], ps[:], SWAP_MASK)
                t1 = rt.tile([128, 512], BF16, tag="t1")
                nc.vector.tensor_tensor(t1[:], ps[:], cos_sb[:, sl], OP.mult)
                t2 = rt.tile([128, 512], BF16, tag="t2")
                nc.gpsimd.tensor_tensor(t2[:], sw[:], sin_sb[:, sl], OP.mult)
                nc.vector.tensor_tensor(outT[:, mt, sl], t1[:], t2[:], OP.add)

        def v_proj():
            for st2 in range(8):
                ps = psa.tile([128, 512], F32, tag="pj", bufs=2)
                for half in range(2):
                    st = 2 * st2 + half
                    for kt in range(8):
                        _mm(nc, ps[:, DPC * half:DPC * (half + 1)],
                            xT_sb[:, kt, 128 * st:128 * (st + 1)],
                            wv_sb[:, kt, :],
                            start=(kt == 0), stop=(kt == 7))
                nc.vector.tensor_copy(
                    v_sb[:, 2 * st2:2 * st2 + 2, :, 0:DK],
                    ps[:].rearrange("p (s h e) -> p s h e", s=2, h=HPC),
                )

        def attention(h):
            sub, ph = h % 2, h // 2
            prow = slice(64 * sub, 64 * (sub + 1))
            for H in range(2):
                q_hi = 1024 * (H + 1)
                at_ps = psa.tile([DK + 1, 1024], F32, tag="at", bufs=1)
                t_hi = 8 * (H + 1)  # exclusive
                for t in range(t_hi):
                    sq_lo = max(128 * t, 1024 * H)
                    L = q_hi - sq_lo
                    sc = psa.tile([128, 1024], F32, tag="sc", bufs=2)
                    off = 0
                    while off < L:
                        n = min(512, L - off)
                        _mm(nc, sc[:, off:off + n],
                            kT_sb[prow, ph, 128 * t:128 * (t + 1)],
                            qT_sb[prow, ph, sq_lo + off:sq_lo + off + n],
                            start=True, stop=True)
                        off += n
                    ex = pe.tile([128, 1024], BF16, tag="exp")
                    nc.scalar.activation(ex[:, 0:L], sc[:, 0:L], AF.Exp, scale=0.125)
                    if 128 * t >= 1024 * H:
                        # diagonal block: zero exp where local_sq < partition
                        nc.gpsimd.tensor_tensor(
                            ex[:, 0:128], ex[:, 0:128], dmask_sb[:], OP.mult)
                    # PV accumulation (+ denominator via ones column)
                    for ck in range(2):
                        c_lo, c_hi = 1024 * H + 512 * ck, 1024 * H + 512 * (ck + 1)
                        if sq_lo >= c_hi:
                            continue
                        lo = max(sq_lo, c_lo)
                        last_t = min(t_hi, (c_hi + 127) // 128) - 1
                        _mm(nc, at_ps[:, lo - 1024 * H:c_hi - 1024 * H],
                            v_sb[:, t, h, :],
                            ex[:, lo - sq_lo:c_hi - sq_lo],
                            start=(t == 0), stop=(t == last_t))
                # normalize: recip of denom row, PE broadcast matmul, multiply
                rc = pa.tile([DK + 1, 1024], BF16, tag="rc")
                with nc.allow_low_precision(reason="bf16 softmax recip broadcast"):
                    nc.vector.reciprocal(rc[DK:DK + 1, :], at_ps[DK:DK + 1, :])
                ac = pa.tile([DK, 1024], BF16, tag="ac")
                nc.scalar.copy(ac[:], at_ps[0:DK, :])
                bc_ps = psa.tile([DK, 1024], F32, tag="sc", bufs=2)
                for bh in range(2):
                    _mm(nc, bc_ps[:, 512 * bh:512 * (bh + 1)],
                        ones_sb[DK:DK + 1, :],
                        rc[DK:DK + 1, 512 * bh:512 * (bh + 1)],
                        start=True, stop=True)
                Hsl = slice(1024 * H, 1024 * (H + 1))
                if sub == 0:
                    nc.vector.tensor_tensor(
                        attnT_sb[0:64, ph, Hsl], ac[:], bc_ps[:], OP.mult)
                else:
                    tn = pa.tile([DK, 1024], BF16, tag="tn")
                    nc.vector.tensor_tensor(tn[:], ac[:], bc_ps[:], OP.mult)
                    nc.sync.dma_start(attnT_sb[64:128, ph, Hsl], tn[:])

        def out_proj():
            optags = [("pj", 2), ("sc", 2), ("at", 1)]
            for st in range(16):
                for ncb in range(2):
                    tg, bf = optags[(2 * st + ncb) % 3]
                    po = psa.tile([128, 512], F32, tag=tg, bufs=bf)
                    for kt2 in range(2):
                        _mm(nc, po[:, 0:512],
                            attnT_sb[:, kt2, 128 * st:128 * (st + 1)],
                            wo_sb[:, kt2, 512 * ncb:512 * (ncb + 1)],
                            start=(kt2 == 0), stop=(kt2 == 1))
                    ob = pa.tile([128, 512], F32, tag="ob", bufs=6)
                    if ncb == 0:
                        nc.scalar.copy(ob[:], po[:, 0:512])
                    else:
                        nc.vector.tensor_copy(ob[:], po[:, 0:512])
                    nc.sync.dma_start(
                        outp[128 * st:128 * (st + 1), 512 * ncb:512 * (ncb + 1)],
                        ob[:])

        # head-pair pipelined emission: attention on heads 0,1 overlaps
        # the projections for heads 2,3
        for c in range(4):
            qk_chunk(wq_sb, qT_sb, 0, c)
            qk_chunk(wk_sb, kT_sb, 0, c)
        v_proj()
        with tc.high_priority():
            attention(0)
            attention(1)
        for c in range(4):
            qk_chunk(wq_sb, qT_sb, 1, c)
            qk_chunk(wk_sb, kT_sb, 1, c)
        attention(3)
        attention(2)
        out_proj()


_CACHE = {}


def _build():
    if "nc" in _CACHE:
        return _CACHE["nc"], _CACHE["aps"]
    nc = bacc.Bacc("TRN2", target_bir_lowering=False, debug=False,
                   enable_asserts=False, num_devices=N_CORES)
    aps = {
        "xT": nc.dram_tensor("xT", [D, S], BF16, kind="ExternalInput").ap(),
        "wqT": nc.dram_tensor("wqT", [D, DPC], BF16, kind="ExternalInput").ap(),
        "wkT": nc.dram_tensor("wkT", [D, DPC], BF16, kind="ExternalInput").ap(),
        "wvT": nc.dram_tensor("wvT", [D, DPC], BF16, kind="ExternalInput").ap(),
        "woT": nc.dram_tensor("woT", [DPC, D], BF16, kind="ExternalInput").ap(),
        "cosT": nc.dram_tensor("cosT", [128, S], F32, kind="ExternalInput").ap(),
        "sinT": nc.dram_tensor("sinT", [128, S], F32, kind="ExternalInput").ap(),
        "dmask": nc.dram_tensor("dmask", [128, 128], BF16, kind="ExternalInput").ap(),
        "out": nc.dram_tensor("out", [S, D], F32, kind="ExternalOutput").ap(),
    }
    with tile.TileContext(nc) as tc:
        _emit(tc, aps)
    nc.compile()
    _CACHE["nc"], _CACHE["aps"] = nc, aps
    return nc, aps


def _host_tables():
    pos = np.arange(S, dtype=np.float64)
    freqs = THETA ** (-np.arange(0, DK, 2, dtype=np.float64) / DK)
    ang = pos[:, None] * freqs[None, :]          # [S, 32]
    cos64 = np.empty((64, S), np.float32)
    sin64 = np.empty((64, S), np.float32)
    cos64[0::2] = cos64[1::2] = np.cos(ang).T
    sin64[0::2] = -np.sin(ang).T
    sin64[1::2] = np.sin(ang).T
    return (np.ascontiguousarray(np.concatenate([cos64, cos64], axis=0)),
            np.ascontiguousarray(np.concatenate([sin64, sin64], axis=0)))


def make_in_maps(x, Wq, Wk, Wv, Wo):
    cosT, sinT = _host_tables()
    dmask = np.triu(np.ones((128, 128), ml_dtypes.bfloat16))  # keep sq >= sk
    xT = [np.ascontiguousarray(x[b].T.astype(ml_dtypes.bfloat16)) for b in range(B)]
    maps = []
    for c in range(N_CORES):
        b, g = c // 4, c % 4
        rows = slice(DPC * g, DPC * (g + 1))
        maps.append({
            "xT": xT[b],
            "wqT": np.ascontiguousarray(Wq[rows, :].T.astype(ml_dtypes.bfloat16)),
            "wkT": np.ascontiguousarray(Wk[rows, :].T.astype(ml_dtypes.bfloat16)),
            "wvT": np.ascontiguousarray(Wv[rows, :].T.astype(ml_dtypes.bfloat16)),
            "woT": np.ascontiguousarray(Wo[:, rows].T.astype(ml_dtypes.bfloat16)),
            "cosT": cosT,
            "sinT": sinT,
            "dmask": dmask,
        })
    return maps


def kernel(x, Wq, Wk, Wv, Wo, _trace=False, _tmpdir=None):
    x, Wq, Wk, Wv, Wo = (np.asarray(a, dtype=np.float32) for a in (x, Wq, Wk, Wv, Wo))
    nc, _ = _build()
    maps = make_in_maps(x, Wq, Wk, Wv, Wo)
    res = run_bass_kernel_spmd(nc, maps, core_ids=list(range(N_CORES)),
                               trace=_trace, tmpdir=_tmpdir)
    out = np.zeros((B, S, D), np.float32)
    for c in range(N_CORES):
        out[c // 4] += res.results[c]["out"]
    if _trace:
        kernel.last_results = res
    return out


# revision 42
# speedup vs baseline: 1.1695x; 1.0028x over previous
"""Multi-head self-attention with RoPE (causal) on 8 Trainium2 NeuronCores.

Sharding: core c -> batch b = c//4, head-group g = c%4 (heads 4g..4g+3).
Each core computes a partial output x[b] @ block of Wo; host sums the 4
partials per batch.

Per-core layout strategy (all matmuls in bf16, fp32 PSUM accumulation):
  - x is fed transposed (xT [1024, 2048]); q,k are produced directly in
    transposed layout qT/kT [256 dims, 2048 seq] (dims on partitions).
  - RoPE applied in transposed layout: pair-swap via DVE stream_shuffle,
    combine with host-precomputed cos / sign-folded sin tables.
  - scores computed transposed (scoresT [sk, sq]) so softmax's key-sum is a
    matmul reduction: a ones-column appended to v makes the PV matmul emit
    the softmax denominator as out row 64.
  - exp on ScalarE with scale=1/8 (the 1/sqrt(d_k)); causal masking by
    computing only sq >= 128*t_sk per key tile + one affine_select zeroing
    of the diagonal 128x128 block after exp.
"""

import ml_dtypes
import numpy as np

import concourse.bass as bass
import concourse.mybir as mybir
import concourse.tile as tile
from concourse import bacc
from concourse import library_config
from concourse.bass_utils import run_bass_kernel_spmd

F32 = mybir.dt.float32
F32R = mybir.dt.float32r
BF16 = mybir.dt.bfloat16

D = 1024          # d_model
NH = 16           # total heads
DK = 64           # head dim
S = 2048          # seq len
B = 2             # batch
THETA = 10000.0
HPC = 4           # heads per core
DPC = HPC * DK    # dims per core = 256
N_CORES = 8

SWAP_MASK = [(i ^ 1) for i in range(32)]  # pair-swap within 32-lane groups


def _mm(nc, out, lhsT, rhs, start, stop):
    nc.tensor.matmul(out, lhsT, rhs, start=start, stop=stop)


def _emit(tc, aps):
    nc = tc.nc
    xT, wq, wk, wv, wo, cosc, sinc, outp = (
        aps["xT"], aps["wqT"], aps["wkT"], aps["wvT"], aps["woT"],
        aps["cosT"], aps["sinT"], aps["out"],
    )
    AF = mybir.ActivationFunctionType
    OP = mybir.AluOpType

    with (
        tc.tile_pool(name="persist", bufs=1) as pp,
        tc.tile_pool(name="ropetmp", bufs=2) as rt,
        tc.tile_pool(name="attn", bufs=2) as pa,
        tc.tile_pool(name="exp", bufs=14) as pe,
        tc.tile_pool(name="psum", bufs=2, space="PSUM") as psa,
    ):
        # ---- persistent SBUF tensors ----
        qT_sb = pp.tile([128, 2, S], BF16, tag="qT")
        kT_sb = pp.tile([128, 2, S], BF16, tag="kT")
        v_sb = pp.tile([128, 16, HPC, DK + 1], BF16, tag="v")
        wo_sb = pp.tile([128, 2, D], BF16, tag="wo")
        attnT_sb = pp.tile([128, 2, S], BF16, tag="attnT")
        dmask_sb = pp.tile([128, 128], BF16, tag="dmask")
        xT_sb = pp.tile([128, 8, S], BF16, tag="xT")
        wq_sb = pp.tile([128, 8, DPC], BF16, tag="wq")
        wk_sb = pp.tile([128, 8, DPC], BF16, tag="wk")
        wv_sb = pp.tile([128, 8, DPC], BF16, tag="wv")
        cos_sb = pp.tile([128, S], F32, tag="cos")
        sin_sb = pp.tile([128, S], F32, tag="sin")

        # input DMAs, ordered so the first projection chunk unblocks ASAP:
        # wq fully, then xT's first 512 columns across all k-tiles
        def xT_dma(c):
            for kt in range(8):
                nc.sync.dma_start(
                    xT_sb[:, kt, 512 * c:512 * (c + 1)],
                    xT[128 * kt:128 * (kt + 1), 512 * c:512 * (c + 1)])
        def cs_dma(c):
            sl = slice(512 * c, 512 * (c + 1))
            nc.sync.dma_start(cos_sb[:, sl], cosc[:, sl])
            nc.sync.dma_start(sin_sb[:, sl], sinc[:, sl])
        for kt in range(8):
            nc.sync.dma_start(wq_sb[:, kt, :], wq[128 * kt:128 * (kt + 1), :])
        xT_dma(0)
        cs_dma(0)
        for kt in range(8):
            nc.sync.dma_start(wk_sb[:, kt, :], wk[128 * kt:128 * (kt + 1), :])
        xT_dma(1)
        cs_dma(1)
        xT_dma(2)
        cs_dma(2)
        xT_dma(3)
        cs_dma(3)
        nc.sync.dma_start(dmask_sb[:], aps["dmask"][:])
        for kt in range(8):
            nc.sync.dma_start(wv_sb[:, kt, :], wv[128 * kt:128 * (kt + 1), :])
        nc.sync.dma_start(wo_sb[:], wo.rearrange("(n p) m -> p n m", p=128))
        # ones column of v (denominator trick)
        nc.gpsimd.memset(v_sb[:, :, :, DK], 1.0)
        ones_sb = pp.tile([DK + 1, DK], BF16, tag="ones")
        nc.gpsimd.memset(ones_sb[:], 1.0)

        def qk_chunk(w_sb, outT, mt, c):
                ps = psa.tile([128, 512], F32, tag="pj", bufs=2)
                for kt in range(8):
                    _mm(nc, ps[:],
                        w_sb[:, kt, 128 * mt:128 * (mt + 1)],
                        xT_sb[:, kt, 512 * c:512 * (c + 1)],
                        start=(kt == 0), stop=(kt == 7))
                sl = slice(512 * c, 512 * (c + 1))
                sw = rt.tile([128, 512], F32, tag="sw")
                nc.vector.stream_shuffle(sw[:], ps[:], SWAP_MASK)
                t1 = rt.tile([128, 512], BF16, tag="t1")
                nc.vector.tensor_tensor(t1[:], ps[:], cos_sb[:, sl], OP.mult)
                t2 = rt.tile([128, 512], BF16, tag="t2")
                nc.gpsimd.tensor_tensor(t2[:], sw[:], sin_sb[:, sl], OP.mult)
                nc.vector.tensor_tensor(outT[:, mt, sl], t1[:], t2[:], OP.add)

        def v_proj():
            for st2 in range(8):
                ps = psa.tile([128, 512], F32, tag="pj", bufs=2)
                for half in range(2):
                    st = 2 * st2 + half
                    for kt in range(8):
                        _mm(nc, ps[:, DPC * half:DPC * (half + 1)],
                            xT_sb[:, kt, 128 * st:128 * (st + 1)],
                            wv_sb[:, kt, :],
                            start=(kt == 0), stop=(kt == 7))
                nc.vector.tensor_copy(
                    v_sb[:, 2 * st2:2 * st2 + 2, :, 0:DK],
                    ps[:].rearrange("p (s h e) -> p s h e", s=2, h=HPC),
                )

        def attention(h):
            sub, ph = h % 2, h // 2
            prow = slice(64 * sub, 64 * (sub + 1))
            for H in range(2):
                q_hi = 1024 * (H + 1)
                at_ps = psa.tile([DK + 1, 1024], F32, tag="at", bufs=1)
                t_hi = 8 * (H + 1)  # exclusive
                for t in range(t_hi):
                    sq_lo = max(128 * t, 1024 * H)
                    L = q_hi - sq_lo
                    sc = psa.tile([128, 1024], F32, tag="sc", bufs=2)
                    off = 0
                    while off < L:
                        n = min(512, L - off)
                        _mm(nc, sc[:, off:off + n],
                            kT_sb[prow, ph, 128 * t:128 * (t + 1)],
                            qT_sb[prow, ph, sq_lo + off:sq_lo + off + n],
                            start=True, stop=True)
                        off += n
                    ex = pe.tile([128, 1024], BF16, tag="exp")
                    nc.scalar.activation(ex[:, 0:L], sc[:, 0:L], AF.Exp, scale=0.125)
                    if 128 * t >= 1024 * H:
                        # diagonal block: zero exp where local_sq < partition
                        nc.gpsimd.tensor_tensor(
                            ex[:, 0:128], ex[:, 0:128], dmask_sb[:], OP.mult)
                    # PV accumulation (+ denominator via ones column)
                    for ck in range(2):
                        c_lo, c_hi = 1024 * H + 512 * ck, 1024 * H + 512 * (ck + 1)
                        if sq_lo >= c_hi:
                            continue
                        lo = max(sq_lo, c_lo)
                        last_t = min(t_hi, (c_hi + 127) // 128) - 1
                        _mm(nc, at_ps[:, lo - 1024 * H:c_hi - 1024 * H],
                            v_sb[:, t, h, :],
                            ex[:, lo - sq_lo:c_hi - sq_lo],
                            start=(t == 0), stop=(t == last_t))
                # normalize, pipelined in 512-col halves: each half's
                # recip/copy/broadcast/multiply starts as soon as that
                # half's last PV lands (chunk A finishes before chunk B)
                rc = pa.tile([DK + 1, 1024], BF16, tag="rc")
                ac = pa.tile([DK, 1024], BF16, tag="ac")
                bc_ps = psa.tile([DK, 1024], F32, tag="sc", bufs=2)
                tn = pa.tile([DK, 1024], BF16, tag="tn")
                for bh in range(2):
                    hsl = slice(512 * bh, 512 * (bh + 1))
                    with nc.allow_low_precision(reason="bf16 softmax recip"):
                        nc.vector.reciprocal(
                            rc[DK:DK + 1, hsl], at_ps[DK:DK + 1, hsl])
                    nc.scalar.copy(ac[:, hsl], at_ps[0:DK, hsl])
                    _mm(nc, bc_ps[:, hsl],
                        ones_sb[DK:DK + 1, :], rc[DK:DK + 1, hsl],
                        start=True, stop=True)
                    osl = slice(1024 * H + 512 * bh, 1024 * H + 512 * (bh + 1))
                    if sub == 0:
                        nc.vector.tensor_tensor(
                            attnT_sb[0:64, ph, osl],
                            ac[:, hsl], bc_ps[:, hsl], OP.mult)
                    else:
                        nc.vector.tensor_tensor(
                            tn[:, hsl], ac[:, hsl], bc_ps[:, hsl], OP.mult)
                        nc.sync.dma_start(
                            attnT_sb[64:128, ph, osl], tn[:, hsl])

        def out_proj():
            optags = [("pj", 2), ("sc", 2), ("at", 1)]
            for st in range(16):
                for ncb in range(2):
                    tg, bf = optags[(2 * st + ncb) % 3]
                    po = psa.tile([128, 512], F32, tag=tg, bufs=bf)
                    for kt2 in range(2):
                        _mm(nc, po[:, 0:512],
                            attnT_sb[:, kt2, 128 * st:128 * (st + 1)],
                            wo_sb[:, kt2, 512 * ncb:512 * (ncb + 1)],
                            start=(kt2 == 0), stop=(kt2 == 1))
                    ob = pa.tile([128, 512], F32, tag="ob", bufs=6)
                    if ncb == 0:
                        nc.scalar.copy(ob[:], po[:, 0:512])
                    else:
                        nc.vector.tensor_copy(ob[:], po[:, 0:512])
                    nc.sync.dma_start(
                        outp[128 * st:128 * (st + 1), 512 * ncb:512 * (ncb + 1)],
                        ob[:])

        # head-pair pipelined emission: attention on heads 0,1 overlaps
        # the projections for heads 2,3
        for c in range(4):
            qk_chunk(wq_sb, qT_sb, 0, c)
            qk_chunk(wk_sb, kT_sb, 0, c)
        v_proj()
        with tc.high_priority():
            attention(0)
            attention(1)
        for c in range(4):
            qk_chunk(wq_sb, qT_sb, 1, c)
            qk_chunk(wk_sb, kT_sb, 1, c)
        attention(3)
        attention(2)
        out_proj()


_CACHE = {}


def _build():
    if "nc" in _CACHE:
        return _CACHE["nc"], _CACHE["aps"]
    nc = bacc.Bacc("TRN2", target_bir_lowering=False, debug=False,
                   enable_asserts=False, num_devices=N_CORES)
    aps = {
        "xT": nc.dram_tensor("xT", [D, S], BF16, kind="ExternalInput").ap(),
        "wqT": nc.dram_tensor("wqT", [D, DPC], BF16, kind="ExternalInput").ap(),
        "wkT": nc.dram_tensor("wkT", [D, DPC], BF16, kind="ExternalInput").ap(),
        "wvT": nc.dram_tensor("wvT", [D, DPC], BF16, kind="ExternalInput").ap(),
        "woT": nc.dram_tensor("woT", [DPC, D], BF16, kind="ExternalInput").ap(),
        "cosT": nc.dram_tensor("cosT", [128, S], F32, kind="ExternalInput").ap(),
        "sinT": nc.dram_tensor("sinT", [128, S], F32, kind="ExternalInput").ap(),
        "dmask": nc.dram_tensor("dmask", [128, 128], BF16, kind="ExternalInput").ap(),
        "out": nc.dram_tensor("out", [S, D], F32, kind="ExternalOutput").ap(),
    }
    with tile.TileContext(nc) as tc:
        _emit(tc, aps)
    nc.compile()
    _CACHE["nc"], _CACHE["aps"] = nc, aps
    return nc, aps


def _host_tables():
    pos = np.arange(S, dtype=np.float64)
    freqs = THETA ** (-np.arange(0, DK, 2, dtype=np.float64) / DK)
    ang = pos[:, None] * freqs[None, :]          # [S, 32]
    cos64 = np.empty((64, S), np.float32)
    sin64 = np.empty((64, S), np.float32)
    cos64[0::2] = cos64[1::2] = np.cos(ang).T
    sin64[0::2] = -np.sin(ang).T
    sin64[1::2] = np.sin(ang).T
    return (np.ascontiguousarray(np.concatenate([cos64, cos64], axis=0)),
            np.ascontiguousarray(np.concatenate([sin64, sin64], axis=0)))


def make_in_maps(x, Wq, Wk, Wv, Wo):
    cosT, sinT = _host_tables()
    dmask = np.triu(np.ones((128, 128), ml_dtypes.bfloat16))  # keep sq >= sk
    xT = [np.ascontiguousarray(x[b].T.astype(ml_dtypes.bfloat16)) for b in range(B)]
    maps = []
    for c in range(N_CORES):
        b, g = c // 4, c % 4
        rows = slice(DPC * g, DPC * (g + 1))
        maps.append({
            "xT": xT[b],
            "wqT": np.ascontiguousarray(Wq[rows, :].T.astype(ml_dtypes.bfloat16)),
            "wkT": np.ascontiguousarray(Wk[rows, :].T.astype(ml_dtypes.bfloat16)),
            "wvT": np.ascontiguousarray(Wv[rows, :].T.astype(ml_dtypes.bfloat16)),
            "woT": np.ascontiguousarray(Wo[:, rows].T.astype(ml_dtypes.bfloat16)),
            "cosT": cosT,
            "sinT": sinT,
            "dmask": dmask,
        })
    return maps


def kernel(x, Wq, Wk, Wv, Wo, _trace=False, _tmpdir=None):
    x, Wq, Wk, Wv, Wo = (np.asarray(a, dtype=np.float32) for a in (x, Wq, Wk, Wv, Wo))
    nc, _ = _build()
    maps = make_in_maps(x, Wq, Wk, Wv, Wo)
    res = run_bass_kernel_spmd(nc, maps, core_ids=list(range(N_CORES)),
                               trace=_trace, tmpdir=_tmpdir)
    out = np.zeros((B, S, D), np.float32)
    for c in range(N_CORES):
        out[c // 4] += res.results[c]["out"]
    if _trace:
        kernel.last_results = res
    return out


# revision 43
# speedup vs baseline: 1.1767x; 1.0061x over previous
"""Multi-head self-attention with RoPE (causal) on 8 Trainium2 NeuronCores.

Sharding: core c -> batch b = c//4, head-group g = c%4 (heads 4g..4g+3).
Each core computes a partial output x[b] @ block of Wo; host sums the 4
partials per batch.

Per-core layout strategy (all matmuls in bf16, fp32 PSUM accumulation):
  - x is fed transposed (xT [1024, 2048]); q,k are produced directly in
    transposed layout qT/kT [256 dims, 2048 seq] (dims on partitions).
  - RoPE applied in transposed layout: pair-swap via DVE stream_shuffle,
    combine with host-precomputed cos / sign-folded sin tables.
  - scores computed transposed (scoresT [sk, sq]) so softmax's key-sum is a
    matmul reduction: a ones-column appended to v makes the PV matmul emit
    the softmax denominator as out row 64.
  - exp on ScalarE with scale=1/8 (the 1/sqrt(d_k)); causal masking by
    computing only sq >= 128*t_sk per key tile + one affine_select zeroing
    of the diagonal 128x128 block after exp.
"""

import ml_dtypes
import numpy as np

import concourse.bass as bass
import concourse.mybir as mybir
import concourse.tile as tile
from concourse import bacc
from concourse import library_config
from concourse.bass_utils import run_bass_kernel_spmd

F32 = mybir.dt.float32
F32R = mybir.dt.float32r
BF16 = mybir.dt.bfloat16

D = 1024          # d_model
NH = 16           # total heads
DK = 64           # head dim
S = 2048          # seq len
B = 2             # batch
THETA = 10000.0
HPC = 4           # heads per core
DPC = HPC * DK    # dims per core = 256
N_CORES = 8

SWAP_MASK = [(i ^ 1) for i in range(32)]  # pair-swap within 32-lane groups


def _mm(nc, out, lhsT, rhs, start, stop):
    nc.tensor.matmul(out, lhsT, rhs, start=start, stop=stop)


def _emit(tc, aps):
    nc = tc.nc
    xT, wq, wk, wv, wo, cosc, sinc, outp = (
        aps["xT"], aps["wqT"], aps["wkT"], aps["wvT"], aps["woT"],
        aps["cosT"], aps["sinT"], aps["out"],
    )
    AF = mybir.ActivationFunctionType
    OP = mybir.AluOpType

    with (
        tc.tile_pool(name="persist", bufs=1) as pp,
        tc.tile_pool(name="ropetmp", bufs=2) as rt,
        tc.tile_pool(name="attn", bufs=2) as pa,
        tc.tile_pool(name="exp", bufs=14) as pe,
        tc.tile_pool(name="psum", bufs=2, space="PSUM") as psa,
    ):
        # ---- persistent SBUF tensors ----
        qT_sb = pp.tile([128, 2, S], BF16, tag="qT")
        kT_sb = pp.tile([128, 2, S], BF16, tag="kT")
        v_sb = pp.tile([128, 16, HPC, DK + 1], BF16, tag="v")
        wo_sb = pp.tile([128, 2, D], BF16, tag="wo")
        attnT_sb = pp.tile([128, 2, S], BF16, tag="attnT")
        dmask_sb = pp.tile([128, 128], BF16, tag="dmask")
        xT_sb = pp.tile([128, 8, S], BF16, tag="xT")
        wq_sb = pp.tile([128, 8, DPC], BF16, tag="wq")
        wk_sb = pp.tile([128, 8, DPC], BF16, tag="wk")
        wv_sb = pp.tile([128, 8, DPC], BF16, tag="wv")
        cos_sb = pp.tile([128, S], F32, tag="cos")
        sin_sb = pp.tile([128, S], F32, tag="sin")

        # input DMAs, ordered so the first projection chunk unblocks ASAP:
        # wq fully, then xT's first 512 columns across all k-tiles
        def xT_dma(c):
            for kt in range(8):
                nc.sync.dma_start(
                    xT_sb[:, kt, 512 * c:512 * (c + 1)],
                    xT[128 * kt:128 * (kt + 1), 512 * c:512 * (c + 1)])
        def cs_dma(c):
            sl = slice(512 * c, 512 * (c + 1))
            nc.sync.dma_start(cos_sb[:, sl], cosc[:, sl])
            nc.sync.dma_start(sin_sb[:, sl], sinc[:, sl])
        for kt in range(8):
            nc.sync.dma_start(wq_sb[:, kt, :], wq[128 * kt:128 * (kt + 1), :])
        xT_dma(0)
        cs_dma(0)
        for kt in range(8):
            nc.sync.dma_start(wk_sb[:, kt, :], wk[128 * kt:128 * (kt + 1), :])
        xT_dma(1)
        cs_dma(1)
        xT_dma(2)
        cs_dma(2)
        xT_dma(3)
        cs_dma(3)
        nc.sync.dma_start(dmask_sb[:], aps["dmask"][:])
        for kt in range(8):
            nc.sync.dma_start(wv_sb[:, kt, :], wv[128 * kt:128 * (kt + 1), :])
        nc.sync.dma_start(wo_sb[:], wo.rearrange("(n p) m -> p n m", p=128))
        # ones column of v (denominator trick)
        nc.gpsimd.memset(v_sb[:, :, :, DK], 1.0)
        ones_sb = pp.tile([DK + 1, DK], BF16, tag="ones")
        nc.gpsimd.memset(ones_sb[:], 1.0)

        def qk_chunk(w_sb, outT, mt, c):
                ps = psa.tile([128, 512], F32, tag="pj", bufs=2)
                for kt in range(8):
                    _mm(nc, ps[:],
                        w_sb[:, kt, 128 * mt:128 * (mt + 1)],
                        xT_sb[:, kt, 512 * c:512 * (c + 1)],
                        start=(kt == 0), stop=(kt == 7))
                sl = slice(512 * c, 512 * (c + 1))
                sw = rt.tile([128, 512], F32, tag="sw")
                nc.vector.stream_shuffle(sw[:], ps[:], SWAP_MASK)
                t1 = rt.tile([128, 512], BF16, tag="t1")
                nc.vector.tensor_tensor(t1[:], ps[:], cos_sb[:, sl], OP.mult)
                t2 = rt.tile([128, 512], BF16, tag="t2")
                nc.gpsimd.tensor_tensor(t2[:], sw[:], sin_sb[:, sl], OP.mult)
                nc.vector.tensor_tensor(outT[:, mt, sl], t1[:], t2[:], OP.add)

        def v_proj():
            for st2 in range(8):
                ps = psa.tile([128, 512], F32, tag="pj", bufs=2)
                for half in range(2):
                    st = 2 * st2 + half
                    for kt in range(8):
                        _mm(nc, ps[:, DPC * half:DPC * (half + 1)],
                            xT_sb[:, kt, 128 * st:128 * (st + 1)],
                            wv_sb[:, kt, :],
                            start=(kt == 0), stop=(kt == 7))
                nc.vector.tensor_copy(
                    v_sb[:, 2 * st2:2 * st2 + 2, :, 0:DK],
                    ps[:].rearrange("p (s h e) -> p s h e", s=2, h=HPC),
                )

        def attention(h):
            sub, ph = h % 2, h // 2
            prow = slice(64 * sub, 64 * (sub + 1))
            for H in range(2):
                q_hi = 1024 * (H + 1)
                at_ps = psa.tile([DK + 1, 1024], F32, tag="at", bufs=1)
                t_hi = 8 * (H + 1)  # exclusive
                for t in range(t_hi):
                    sq_lo = max(128 * t, 1024 * H)
                    L = q_hi - sq_lo
                    sc = psa.tile([128, 1024], F32, tag="sc", bufs=2)
                    off = 0
                    while off < L:
                        n = min(512, L - off)
                        _mm(nc, sc[:, off:off + n],
                            kT_sb[prow, ph, 128 * t:128 * (t + 1)],
                            qT_sb[prow, ph, sq_lo + off:sq_lo + off + n],
                            start=True, stop=True)
                        off += n
                    ex = pe.tile([128, 1024], BF16, tag="exp")
                    nc.scalar.activation(ex[:, 0:L], sc[:, 0:L], AF.Exp, scale=0.125)
                    if 128 * t >= 1024 * H:
                        # diagonal block: zero exp where local_sq < partition
                        nc.gpsimd.tensor_tensor(
                            ex[:, 0:128], ex[:, 0:128], dmask_sb[:], OP.mult)
                    # PV accumulation (+ denominator via ones column)
                    for ck in range(2):
                        c_lo, c_hi = 1024 * H + 512 * ck, 1024 * H + 512 * (ck + 1)
                        if sq_lo >= c_hi:
                            continue
                        lo = max(sq_lo, c_lo)
                        last_t = min(t_hi, (c_hi + 127) // 128) - 1
                        _mm(nc, at_ps[:, lo - 1024 * H:c_hi - 1024 * H],
                            v_sb[:, t, h, :],
                            ex[:, lo - sq_lo:c_hi - sq_lo],
                            start=(t == 0), stop=(t == last_t))
                # normalize, pipelined in 512-col halves: each half's
                # recip/copy/broadcast/multiply starts as soon as that
                # half's last PV lands (chunk A finishes before chunk B)
                rc = pa.tile([DK + 1, 1024], BF16, tag="rc")
                ac = pa.tile([DK, 1024], BF16, tag="ac")
                bc_ps = psa.tile([DK, 1024], F32, tag="sc", bufs=2)
                tn = pa.tile([DK, 1024], BF16, tag="tn")
                for bh in range(2):
                    hsl = slice(512 * bh, 512 * (bh + 1))
                    with nc.allow_low_precision(reason="bf16 softmax recip"):
                        nc.vector.reciprocal(
                            rc[DK:DK + 1, hsl], at_ps[DK:DK + 1, hsl])
                    nc.scalar.copy(ac[:, hsl], at_ps[0:DK, hsl])
                    _mm(nc, bc_ps[:, hsl],
                        ones_sb[DK:DK + 1, :], rc[DK:DK + 1, hsl],
                        start=True, stop=True)
                    osl = slice(1024 * H + 512 * bh, 1024 * H + 512 * (bh + 1))
                    if sub == 0:
                        nc.vector.tensor_tensor(
                            attnT_sb[0:64, ph, osl],
                            ac[:, hsl], bc_ps[:, hsl], OP.mult)
                    else:
                        nc.vector.tensor_tensor(
                            tn[:, hsl], ac[:, hsl], bc_ps[:, hsl], OP.mult)
                        nc.sync.dma_start(
                            attnT_sb[64:128, ph, osl], tn[:, hsl])

        def out_proj():
            optags = [("pj", 2), ("sc", 2), ("at", 1)]
            for st in range(16):
                for ncb in range(2):
                    tg, bf = optags[(2 * st + ncb) % 3]
                    po = psa.tile([128, 512], F32, tag=tg, bufs=bf)
                    for kt2 in range(2):
                        _mm(nc, po[:, 0:512],
                            attnT_sb[:, kt2, 128 * st:128 * (st + 1)],
                            wo_sb[:, kt2, 512 * ncb:512 * (ncb + 1)],
                            start=(kt2 == 0), stop=(kt2 == 1))
                    ob = pa.tile([128, 512], BF16, tag="ob", bufs=6)
                    if ncb == 0:
                        nc.scalar.copy(ob[:], po[:, 0:512])
                    else:
                        nc.vector.tensor_copy(ob[:], po[:, 0:512])
                    nc.sync.dma_start(
                        outp[128 * st:128 * (st + 1), 512 * ncb:512 * (ncb + 1)],
                        ob[:])

        # head-pair pipelined emission: attention on heads 0,1 overlaps
        # the projections for heads 2,3
        for c in range(4):
            qk_chunk(wq_sb, qT_sb, 0, c)
            qk_chunk(wk_sb, kT_sb, 0, c)
        v_proj()
        with tc.high_priority():
            attention(0)
            attention(1)
        for c in range(4):
            qk_chunk(wq_sb, qT_sb, 1, c)
            qk_chunk(wk_sb, kT_sb, 1, c)
        attention(3)
        attention(2)
        out_proj()


_CACHE = {}


def _build():
    if "nc" in _CACHE:
        return _CACHE["nc"], _CACHE["aps"]
    nc = bacc.Bacc("TRN2", target_bir_lowering=False, debug=False,
                   enable_asserts=False, num_devices=N_CORES)
    aps = {
        "xT": nc.dram_tensor("xT", [D, S], BF16, kind="ExternalInput").ap(),
        "wqT": nc.dram_tensor("wqT", [D, DPC], BF16, kind="ExternalInput").ap(),
        "wkT": nc.dram_tensor("wkT", [D, DPC], BF16, kind="ExternalInput").ap(),
        "wvT": nc.dram_tensor("wvT", [D, DPC], BF16, kind="ExternalInput").ap(),
        "woT": nc.dram_tensor("woT", [DPC, D], BF16, kind="ExternalInput").ap(),
        "cosT": nc.dram_tensor("cosT", [128, S], F32, kind="ExternalInput").ap(),
        "sinT": nc.dram_tensor("sinT", [128, S], F32, kind="ExternalInput").ap(),
        "dmask": nc.dram_tensor("dmask", [128, 128], BF16, kind="ExternalInput").ap(),
        "out": nc.dram_tensor("out", [S, D], BF16, kind="ExternalOutput").ap(),
    }
    with tile.TileContext(nc) as tc:
        _emit(tc, aps)
    nc.compile()
    _CACHE["nc"], _CACHE["aps"] = nc, aps
    return nc, aps


def _host_tables():
    pos = np.arange(S, dtype=np.float64)
    freqs = THETA ** (-np.arange(0, DK, 2, dtype=np.float64) / DK)
    ang = pos[:, None] * freqs[None, :]          # [S, 32]
    cos64 = np.empty((64, S), np.float32)
    sin64 = np.empty((64, S), np.float32)
    cos64[0::2] = cos64[1::2] = np.cos(ang).T
    sin64[0::2] = -np.sin(ang).T
    sin64[1::2] = np.sin(ang).T
    return (np.ascontiguousarray(np.concatenate([cos64, cos64], axis=0)),
            np.ascontiguousarray(np.concatenate([sin64, sin64], axis=0)))


def make_in_maps(x, Wq, Wk, Wv, Wo):
    cosT, sinT = _host_tables()
    dmask = np.triu(np.ones((128, 128), ml_dtypes.bfloat16))  # keep sq >= sk
    xT = [np.ascontiguousarray(x[b].T.astype(ml_dtypes.bfloat16)) for b in range(B)]
    maps = []
    for c in range(N_CORES):
        b, g = c // 4, c % 4
        rows = slice(DPC * g, DPC * (g + 1))
        maps.append({
            "xT": xT[b],
            "wqT": np.ascontiguousarray(Wq[rows, :].T.astype(ml_dtypes.bfloat16)),
            "wkT": np.ascontiguousarray(Wk[rows, :].T.astype(ml_dtypes.bfloat16)),
            "wvT": np.ascontiguousarray(Wv[rows, :].T.astype(ml_dtypes.bfloat16)),
            "woT": np.ascontiguousarray(Wo[:, rows].T.astype(ml_dtypes.bfloat16)),
            "cosT": cosT,
            "sinT": sinT,
            "dmask": dmask,
        })
    return maps


def kernel(x, Wq, Wk, Wv, Wo, _trace=False, _tmpdir=None):
    x, Wq, Wk, Wv, Wo = (np.asarray(a, dtype=np.float32) for a in (x, Wq, Wk, Wv, Wo))
    nc, _ = _build()
    maps = make_in_maps(x, Wq, Wk, Wv, Wo)
    res = run_bass_kernel_spmd(nc, maps, core_ids=list(range(N_CORES)),
                               trace=_trace, tmpdir=_tmpdir)
    out = np.zeros((B, S, D), np.float32)
    for c in range(N_CORES):
        out[c // 4] += res.results[c]["out"].astype(np.float32)
    if _trace:
        kernel.last_results = res
    return out


# revision 44
# speedup vs baseline: 1.2121x; 1.0301x over previous
"""Multi-head self-attention with RoPE (causal) on 8 Trainium2 NeuronCores.

Sharding: core c -> batch b = c//4, head-group g = c%4 (heads 4g..4g+3).
Each core computes a partial output x[b] @ block of Wo; host sums the 4
partials per batch.

Per-core layout strategy (all matmuls in bf16, fp32 PSUM accumulation):
  - x is fed transposed (xT [1024, 2048]); q,k are produced directly in
    transposed layout qT/kT [256 dims, 2048 seq] (dims on partitions).
  - RoPE applied in transposed layout: pair-swap via DVE stream_shuffle,
    combine with host-precomputed cos / sign-folded sin tables.
  - scores computed transposed (scoresT [sk, sq]) so softmax's key-sum is a
    matmul reduction: a ones-column appended to v makes the PV matmul emit
    the softmax denominator as out row 64.
  - exp on ScalarE with scale=1/8 (the 1/sqrt(d_k)); causal masking by
    computing only sq >= 128*t_sk per key tile + one affine_select zeroing
    of the diagonal 128x128 block after exp.
"""

import ml_dtypes
import numpy as np

import concourse.bass as bass
import concourse.mybir as mybir
import concourse.tile as tile
from concourse import bacc
from concourse import library_config
from concourse.bass_utils import run_bass_kernel_spmd

F32 = mybir.dt.float32
F32R = mybir.dt.float32r
BF16 = mybir.dt.bfloat16

D = 1024          # d_model
NH = 16           # total heads
DK = 64           # head dim
S = 2048          # seq len
B = 2             # batch
THETA = 10000.0
HPC = 4           # heads per core
DPC = HPC * DK    # dims per core = 256
N_CORES = 8

SWAP_MASK = [(i ^ 1) for i in range(32)]  # pair-swap within 32-lane groups


def _mm(nc, out, lhsT, rhs, start, stop):
    nc.tensor.matmul(out, lhsT, rhs, start=start, stop=stop)


def _emit(tc, aps):
    nc = tc.nc
    xT, wq, wk, wv, wo, cosc, sinc, outp = (
        aps["xT"], aps["wqT"], aps["wkT"], aps["wvT"], aps["woT"],
        aps["cosT"], aps["sinT"], aps["out"],
    )
    AF = mybir.ActivationFunctionType
    OP = mybir.AluOpType

    with (
        tc.tile_pool(name="persist", bufs=1) as pp,
        tc.tile_pool(name="ropetmp", bufs=4) as rt,
        tc.tile_pool(name="attn", bufs=2) as pa,
        tc.tile_pool(name="exp", bufs=14) as pe,
        tc.tile_pool(name="psum", bufs=2, space="PSUM") as psa,
    ):
        # ---- persistent SBUF tensors ----
        qT_sb = pp.tile([128, 2, S], BF16, tag="qT")
        kT_sb = pp.tile([128, 2, S], BF16, tag="kT")
        v_sb = pp.tile([128, 16, HPC, DK + 1], BF16, tag="v")
        wo_sb = pp.tile([128, 2, D], BF16, tag="wo")
        attnT_sb = pp.tile([128, 2, S], BF16, tag="attnT")
        dmask_sb = pp.tile([128, 128], BF16, tag="dmask")
        xT_sb = pp.tile([128, 8, S], BF16, tag="xT")
        wq_sb = pp.tile([128, 8, DPC], BF16, tag="wq")
        wk_sb = pp.tile([128, 8, DPC], BF16, tag="wk")
        wv_sb = pp.tile([128, 8, DPC], BF16, tag="wv")
        cos_sb = pp.tile([128, S], F32, tag="cos")
        sin_sb = pp.tile([128, S], F32, tag="sin")

        # input DMAs, ordered so the first projection chunk unblocks ASAP:
        # wq fully, then xT's first 512 columns across all k-tiles
        def xT_dma(c):
            for kt in range(8):
                nc.sync.dma_start(
                    xT_sb[:, kt, 512 * c:512 * (c + 1)],
                    xT[128 * kt:128 * (kt + 1), 512 * c:512 * (c + 1)])
        def cs_dma(c):
            sl = slice(512 * c, 512 * (c + 1))
            nc.sync.dma_start(cos_sb[:, sl], cosc[:, sl])
            nc.sync.dma_start(sin_sb[:, sl], sinc[:, sl])
        for kt in range(8):
            nc.sync.dma_start(wq_sb[:, kt, :], wq[128 * kt:128 * (kt + 1), :])
        xT_dma(0)
        cs_dma(0)
        for kt in range(8):
            nc.sync.dma_start(wk_sb[:, kt, :], wk[128 * kt:128 * (kt + 1), :])
        xT_dma(1)
        cs_dma(1)
        xT_dma(2)
        cs_dma(2)
        xT_dma(3)
        cs_dma(3)
        nc.sync.dma_start(dmask_sb[:], aps["dmask"][:])
        for kt in range(8):
            nc.sync.dma_start(wv_sb[:, kt, :], wv[128 * kt:128 * (kt + 1), :])
        nc.sync.dma_start(wo_sb[:], wo.rearrange("(n p) m -> p n m", p=128))
        # ones column of v (denominator trick)
        nc.gpsimd.memset(v_sb[:, :, :, DK], 1.0)
        ones_sb = pp.tile([DK + 1, DK], BF16, tag="ones")
        nc.gpsimd.memset(ones_sb[:], 1.0)

        def qk_chunk(w_sb, outT, mt, c):
                ps = psa.tile([128, 512], F32, tag="pj", bufs=2)
                for kt in range(8):
                    _mm(nc, ps[:],
                        w_sb[:, kt, 128 * mt:128 * (mt + 1)],
                        xT_sb[:, kt, 512 * c:512 * (c + 1)],
                        start=(kt == 0), stop=(kt == 7))
                sl = slice(512 * c, 512 * (c + 1))
                sw = rt.tile([128, 512], F32, tag="sw")
                nc.vector.stream_shuffle(sw[:], ps[:], SWAP_MASK)
                t1 = rt.tile([128, 512], BF16, tag="t1")
                nc.vector.tensor_tensor(t1[:], ps[:], cos_sb[:, sl], OP.mult)
                t2 = rt.tile([128, 512], BF16, tag="t2")
                nc.gpsimd.tensor_tensor(t2[:], sw[:], sin_sb[:, sl], OP.mult)
                nc.vector.tensor_tensor(outT[:, mt, sl], t1[:], t2[:], OP.add)

        def v_proj():
            for st2 in range(8):
                ps = psa.tile([128, 512], F32, tag="pj", bufs=2)
                for half in range(2):
                    st = 2 * st2 + half
                    for kt in range(8):
                        _mm(nc, ps[:, DPC * half:DPC * (half + 1)],
                            xT_sb[:, kt, 128 * st:128 * (st + 1)],
                            wv_sb[:, kt, :],
                            start=(kt == 0), stop=(kt == 7))
                nc.vector.tensor_copy(
                    v_sb[:, 2 * st2:2 * st2 + 2, :, 0:DK],
                    ps[:].rearrange("p (s h e) -> p s h e", s=2, h=HPC),
                )

        def attention(h):
            sub, ph = h % 2, h // 2
            prow = slice(64 * sub, 64 * (sub + 1))
            for H in range(2):
                q_hi = 1024 * (H + 1)
                at_ps = psa.tile([DK + 1, 1024], F32, tag="at", bufs=1)
                t_hi = 8 * (H + 1)  # exclusive
                for t in range(t_hi):
                    sq_lo = max(128 * t, 1024 * H)
                    L = q_hi - sq_lo
                    sc = psa.tile([128, 1024], F32, tag="sc", bufs=2)
                    off = 0
                    while off < L:
                        n = min(512, L - off)
                        _mm(nc, sc[:, off:off + n],
                            kT_sb[prow, ph, 128 * t:128 * (t + 1)],
                            qT_sb[prow, ph, sq_lo + off:sq_lo + off + n],
                            start=True, stop=True)
                        off += n
                    ex = pe.tile([128, 1024], BF16, tag="exp")
                    nc.scalar.activation(ex[:, 0:L], sc[:, 0:L], AF.Exp, scale=0.125)
                    if 128 * t >= 1024 * H:
                        # diagonal block: zero exp where local_sq < partition
                        nc.gpsimd.tensor_tensor(
                            ex[:, 0:128], ex[:, 0:128], dmask_sb[:], OP.mult)
                    # PV accumulation (+ denominator via ones column)
                    for ck in range(2):
                        c_lo, c_hi = 1024 * H + 512 * ck, 1024 * H + 512 * (ck + 1)
                        if sq_lo >= c_hi:
                            continue
                        lo = max(sq_lo, c_lo)
                        last_t = min(t_hi, (c_hi + 127) // 128) - 1
                        _mm(nc, at_ps[:, lo - 1024 * H:c_hi - 1024 * H],
                            v_sb[:, t, h, :],
                            ex[:, lo - sq_lo:c_hi - sq_lo],
                            start=(t == 0), stop=(t == last_t))
                # normalize, pipelined in 512-col halves: each half's
                # recip/copy/broadcast/multiply starts as soon as that
                # half's last PV lands (chunk A finishes before chunk B)
                rc = pa.tile([DK + 1, 1024], BF16, tag="rc")
                ac = pa.tile([DK, 1024], BF16, tag="ac")
                bc_ps = psa.tile([DK, 1024], F32, tag="sc", bufs=2)
                tn = pa.tile([DK, 1024], BF16, tag="tn")
                for bh in range(2):
                    hsl = slice(512 * bh, 512 * (bh + 1))
                    with nc.allow_low_precision(reason="bf16 softmax recip"):
                        nc.vector.reciprocal(
                            rc[DK:DK + 1, hsl], at_ps[DK:DK + 1, hsl])
                    nc.scalar.copy(ac[:, hsl], at_ps[0:DK, hsl])
                    _mm(nc, bc_ps[:, hsl],
                        ones_sb[DK:DK + 1, :], rc[DK:DK + 1, hsl],
                        start=True, stop=True)
                    osl = slice(1024 * H + 512 * bh, 1024 * H + 512 * (bh + 1))
                    if sub == 0:
                        nc.vector.tensor_tensor(
                            attnT_sb[0:64, ph, osl],
                            ac[:, hsl], bc_ps[:, hsl], OP.mult)
                    else:
                        nc.vector.tensor_tensor(
                            tn[:, hsl], ac[:, hsl], bc_ps[:, hsl], OP.mult)
                        nc.sync.dma_start(
                            attnT_sb[64:128, ph, osl], tn[:, hsl])

        def out_proj():
            optags = [("pj", 2), ("sc", 2), ("at", 1)]
            for st in range(16):
                ob = pa.tile([128, 1024], BF16, tag="ob", bufs=4)
                for ncb in range(2):
                    tg, bf = optags[(2 * st + ncb) % 3]
                    po = psa.tile([128, 512], F32, tag=tg, bufs=bf)
                    for kt2 in range(2):
                        _mm(nc, po[:, 0:512],
                            attnT_sb[:, kt2, 128 * st:128 * (st + 1)],
                            wo_sb[:, kt2, 512 * ncb:512 * (ncb + 1)],
                            start=(kt2 == 0), stop=(kt2 == 1))
                    if ncb == 0:
                        nc.scalar.copy(ob[:, 0:512], po[:, 0:512])
                    else:
                        nc.vector.tensor_copy(ob[:, 512:1024], po[:, 0:512])
                nc.sync.dma_start(
                    outp[128 * st:128 * (st + 1), :], ob[:])

        # head-pair pipelined emission: attention on heads 0,1 overlaps
        # the projections for heads 2,3
        for c in range(4):
            qk_chunk(wq_sb, qT_sb, 0, c)
            qk_chunk(wk_sb, kT_sb, 0, c)
        v_proj()
        with tc.high_priority():
            attention(0)
            attention(1)
        for c in range(4):
            qk_chunk(wq_sb, qT_sb, 1, c)
            qk_chunk(wk_sb, kT_sb, 1, c)
        attention(3)
        attention(2)
        out_proj()


_CACHE = {}


def _build():
    if "nc" in _CACHE:
        return _CACHE["nc"], _CACHE["aps"]
    nc = bacc.Bacc("TRN2", target_bir_lowering=False, debug=False,
                   enable_asserts=False, num_devices=N_CORES)
    aps = {
        "xT": nc.dram_tensor("xT", [D, S], BF16, kind="ExternalInput").ap(),
        "wqT": nc.dram_tensor("wqT", [D, DPC], BF16, kind="ExternalInput").ap(),
        "wkT": nc.dram_tensor("wkT", [D, DPC], BF16, kind="ExternalInput").ap(),
        "wvT": nc.dram_tensor("wvT", [D, DPC], BF16, kind="ExternalInput").ap(),
        "woT": nc.dram_tensor("woT", [DPC, D], BF16, kind="ExternalInput").ap(),
        "cosT": nc.dram_tensor("cosT", [128, S], F32, kind="ExternalInput").ap(),
        "sinT": nc.dram_tensor("sinT", [128, S], F32, kind="ExternalInput").ap(),
        "dmask": nc.dram_tensor("dmask", [128, 128], BF16, kind="ExternalInput").ap(),
        "out": nc.dram_tensor("out", [S, D], BF16, kind="ExternalOutput").ap(),
    }
    with tile.TileContext(nc) as tc:
        _emit(tc, aps)
    nc.compile()
    _CACHE["nc"], _CACHE["aps"] = nc, aps
    return nc, aps


def _host_tables():
    pos = np.arange(S, dtype=np.float64)
    freqs = THETA ** (-np.arange(0, DK, 2, dtype=np.float64) / DK)
    ang = pos[:, None] * freqs[None, :]          # [S, 32]
    cos64 = np.empty((64, S), np.float32)
    sin64 = np.empty((64, S), np.float32)
    cos64[0::2] = cos64[1::2] = np.cos(ang).T
    sin64[0::2] = -np.sin(ang).T
    sin64[1::2] = np.sin(ang).T
    return (np.ascontiguousarray(np.concatenate([cos64, cos64], axis=0)),
            np.ascontiguousarray(np.concatenate([sin64, sin64], axis=0)))


def make_in_maps(x, Wq, Wk, Wv, Wo):
    cosT, sinT = _host_tables()
    dmask = np.triu(np.ones((128, 128), ml_dtypes.bfloat16))  # keep sq >= sk
    xT = [np.ascontiguousarray(x[b].T.astype(ml_dtypes.bfloat16)) for b in range(B)]
    maps = []
    for c in range(N_CORES):
        b, g = c // 4, c % 4
        rows = slice(DPC * g, DPC * (g + 1))
        maps.append({
            "xT": xT[b],
            "wqT": np.ascontiguousarray(Wq[rows, :].T.astype(ml_dtypes.bfloat16)),
            "wkT": np.ascontiguousarray(Wk[rows, :].T.astype(ml_dtypes.bfloat16)),
            "wvT": np.ascontiguousarray(Wv[rows, :].T.astype(ml_dtypes.bfloat16)),
            "woT": np.ascontiguousarray(Wo[:, rows].T.astype(ml_dtypes.bfloat16)),
            "cosT": cosT,
            "sinT": sinT,
            "dmask": dmask,
        })
    return maps


def kernel(x, Wq, Wk, Wv, Wo, _trace=False, _tmpdir=None):
    x, Wq, Wk, Wv, Wo = (np.asarray(a, dtype=np.float32) for a in (x, Wq, Wk, Wv, Wo))
    nc, _ = _build()
    maps = make_in_maps(x, Wq, Wk, Wv, Wo)
    res = run_bass_kernel_spmd(nc, maps, core_ids=list(range(N_CORES)),
                               trace=_trace, tmpdir=_tmpdir)
    out = np.zeros((B, S, D), np.float32)
    for c in range(N_CORES):
        out[c // 4] += res.results[c]["out"].astype(np.float32)
    if _trace:
        kernel.last_results = res
    return out
